# revision 14
# baseline (speedup 1.0000x reference)
"""DFlashAttention kernel for 8 TRN2 NeuronCores (Bass/Tile).

Sharding: tensor-parallel over heads. Core c owns query heads {2c, 2c+1}
and kv head c (GQA group). Each core computes its heads' full attention
and an o_proj partial; the host sums the 8 partials (the "all-reduce")
and assembles the updated KV caches from the per-core k/v projections.

Device layout notes:
  - x_new is fed pre-transposed (d, t) so projection matmuls can contract
    over d on the partition axis without any on-chip transpose of x.
  - q/k are computed in (t, h) layout (RMSNorm reduces over h on the free
    axis), then PE-transposed to (h, t) for the score matmuls.
  - scores are computed as scoresT[kv, q] so that exp(scores) tiles can be
    used directly as the moving operand of the attention matmul
    (attnT[h, q] += V[kv, h].T @ expT[kv, q]) and the softmax denominator
    is a ones-weight matmul (denom[1, q] += ones[kv, 1].T @ expT[kv, q]).
  - attnT[h, q] is already the stationary-operand layout for o_proj.
"""

import numpy as np

import concourse.bass as bass
import concourse.mybir as mybir
import concourse.tile as tile
from concourse.masks import make_identity
from concourse.vector_clock import ScopedClock

F32 = mybir.dt.float32
F32R = mybir.dt.float32r


class SplitDrainTileContext(tile.TileContext):
    """TileContext that caps every instruction at one sem wait.

    The walrus build in this container rejects instructions with more than
    one sync-wait command (CoreV3GenImpl setupSyncWait: "Too many sync wait
    commands"). Tile routinely assigns several waits to one instruction.
    Semantics are preserved by moving all but the last wait onto same-engine
    NoOps inserted immediately before the instruction — each engine
    sequencer executes its queue in order, so waiting serially on N nops is
    equivalent to one instruction waiting on N semaphores.
    """

    _N_SPLIT_NOPS = 40

    def _add_instruction(self, inst):
        si = inst.sync_info
        if si is not None and si.on_wait is not None and len(si.on_wait) > 1:
            import bass_rust
            waits = list(si.on_wait)
            for w in waits[:-1]:
                nop = bass_rust.InstNoOp(
                    name=self.nc.get_next_instruction_name(), ins=[], outs=[])
                nop.engine = inst.engine
                nop.sync_info = bass_rust.SyncInfo(on_wait=[w], on_update=[])
                super()._add_instruction(nop)
            inst.sync_info = bass_rust.SyncInfo(
                on_wait=[waits[-1]], on_update=list(si.on_update or []))
        super()._add_instruction(inst)

    def _drain_and_barrier(self, tick_clock, wait_clock):
        nops = [
            self.nc.sync.nop(nofuse=True, hint="tail_drain_wait")
            for _ in range(self._N_SPLIT_NOPS)
        ]
        drain_inst = self.nc.sync.drain()
        wait_clock.add_sem_waits(
            drain_inst.ins, ScopedClock({None: tick_clock.global_clock})
        )
        si = drain_inst.ins.sync_info
        waits = list(si.on_wait or [])
        if len(waits) > 1:
            assert len(waits) <= self._N_SPLIT_NOPS + 1, len(waits)
            import bass_rust
            for nop, w in zip(nops, waits[:-1]):
                nop.ins.sync_info = bass_rust.SyncInfo(on_wait=[w], on_update=[])
            drain_inst.ins.sync_info = bass_rust.SyncInfo(
                on_wait=[waits[-1]], on_update=list(si.on_update or []))

        self.nc.all_engine_barrier()
        assert self.sems is not None
        popped = self.nc._tile_sem_poison_stack.pop()
        assert popped is self._sem_poison
        self.nc.clear_and_free_semaphores(list(self.sems.allocated().values()))
        self.nc.all_engine_barrier()

# Problem constants (hardcoded; see module docstring).
D = 2048          # hidden size
N_HEADS = 16      # query heads
K_HEADS = 8       # kv heads
H = 128           # head dim
HALF = H // 2
T_NOISE = 1024
T_PAD = 1024
T_X = T_PAD + T_NOISE          # rows of x_new
MAX_KV = 4096
THETA = 1000000.0
EPS = 1e-6
SM_SCALE = H ** -0.5
N_CORES = 8
P = 128           # partitions

NTT = T_X // P                 # 16 t-tiles over x_new
NQT = T_NOISE // P             # 8 t-tiles over x_noise


def build_bass(n_old: int, n_ctx: int, n_noise: int) -> bass.Bass:
    """Build the per-core Bass module (SPMD: same IR on all 8 cores).

    n_old   = cache_len // 128        (old-cache kv chunks)
    n_ctx   = actual_ctx_count // 128 (valid ctx kv chunks)
    n_noise = T_NOISE // 128          (noise kv chunks)
    """
    nc = bass.Bass()

    # ---- I/O ----
    xT_d = nc.declare_dram_parameter("xT", (D, T_X), F32R, isOutput=False)
    w_all_d = nc.declare_dram_parameter("w_all", (D, 512), F32R, isOutput=False)
    wo_d = nc.declare_dram_parameter("wo", (256, D), F32R, isOutput=False)
    ktold_d = nc.declare_dram_parameter(
        "ktold", (2, H, n_old * P), F32R, isOutput=False)
    vold_d = nc.declare_dram_parameter(
        "vold", (2, n_old * P, H), F32R, isOutput=False)
    cosk_d = nc.declare_dram_parameter("cosk", (T_X, H), F32, isOutput=False)
    sink_d = nc.declare_dram_parameter("sink", (T_X, H), F32, isOutput=False)
    kscale_d = nc.declare_dram_parameter("kscale_b", (P, H), F32, isOutput=False)
    qscale_d = nc.declare_dram_parameter("qscale_b", (P, H), F32, isOutput=False)

    out_d = nc.declare_dram_parameter("out_part", (T_NOISE, D), F32, isOutput=True)
    kout_d = nc.declare_dram_parameter("k_out", (T_X, H), F32, isOutput=True)
    vout_d = nc.declare_dram_parameter("v_out", (T_X, H), F32, isOutput=True)

    n_kv = n_old + n_ctx + n_noise   # total kv chunks (22)

    with SplitDrainTileContext(nc) as tc:
        with tc.tile_pool(name="persist", bufs=1) as ps:
            # ---- persistent SBUF tiles ----
            ident = ps.tile([P, P], F32)
            make_identity(nc, ident)
            ones_f32 = ps.tile([P, P], F32)
            nc.vector.memset(ones_f32, 1.0)
            ones_col = ps.tile([P, 1], F32R)
            nc.vector.tensor_copy(ones_col, ones_f32[:, 0:1])
            ones_row = ps.tile([1, P], F32R)
            nc.vector.tensor_copy(ones_row, ones_f32[0:1, :])
            eps_t = ps.tile([P, 1], F32)
            nc.vector.memset(eps_t, EPS)

            kscale_sb = ps.tile([P, H], F32)
            nc.sync.dma_start(out=kscale_sb, in_=kscale_d[:, :])
            qscale_sb = ps.tile([P, H], F32)
            nc.sync.dma_start(out=qscale_sb, in_=qscale_d[:, :])

            kn = ps.tile([P, NTT, H], F32)       # roped+normed k (t, h)
            v_sb = ps.tile([P, NTT, H], F32)     # raw v (t, h), exact for v_out
            v_r = ps.tile([P, NTT, H], F32R)    # rounded v for matmul
            qn = ps.tile([P, NQT, 256], F32)     # roped+normed q (t, 2 heads)
            kT = ps.tile([P, T_X], F32R)          # k transposed (h, t)
            qT = [ps.tile([P, T_NOISE], F32R, tag=f"qT{j}", name=f"qT{j}") for j in range(2)]
            attnT = [ps.tile([P, 2, 512], F32R, tag=f"attnT{j}", name=f"attnT{j}") for j in range(2)]

            # ---- phase 1: projections q|k|v = x_new @ W ----
            with tc.tile_pool(name="wp", bufs=4) as wp, \
                 tc.tile_pool(name="scr", bufs=1) as scr:
                kn_pre = scr.tile([P, NTT, H], F32)
                qn_pre = scr.tile([P, NQT, 256], F32)
                f_all = scr.tile([P, 64], F32)
                fk = f_all[:, 0:16]
                fq = f_all[:, 16:32]
                fks = f_all[:, 32:48]
                fqs = f_all[:, 48:64]

                # phase 1a: ctx rows (t 0:1024) -> k|v only
                with tc.tile_pool(name="xh", bufs=6) as xh, \
                     tc.tile_pool(name="ppa", bufs=8, space="PSUM") as ppa, \
                     tc.tile_pool(name="sq", bufs=2) as sqp:
                    pa = [ppa.tile([P, 256], F32, tag="pa", name="pa") for _ in range(8)]
                    for dd in range(16):
                        xa = xh.tile([P, T_PAD], F32R, tag="x")
                        nc.sync.dma_start(
                            out=xa, in_=xT_d[dd * P:(dd + 1) * P, 0:T_PAD])
                        wa = wp.tile([P, 256], F32R, tag="wkv", name="wa")
                        nc.sync.dma_start(
                            out=wa, in_=w_all_d[dd * P:(dd + 1) * P, 256:512])
                        for tt in range(8):
                            nc.tensor.matmul(
                                pa[tt],
                                lhsT=xa[:, tt * P:(tt + 1) * P],
                                rhs=wa,
                                start=(dd == 0), stop=(dd == 15))
                    # drain: rmsnorm stats + v copies for ctx tiles
                    for tt in range(8):
                        k_raw = pa[tt][:, 0:H]
                        v_raw = pa[tt][:, H:256]
                        sq = sqp.tile([P, H], F32, tag="sq")
                        nc.scalar.activation(
                            out=sq, in_=k_raw,
                            func=mybir.ActivationFunctionType.Square,
                            accum_out=fk[:, tt:tt + 1])
                        nc.scalar.copy(out=v_sb[:, tt, :], in_=v_raw)
                        nc.vector.tensor_copy(v_r[:, tt, :], v_raw)
                    nc.scalar.activation(
                        out=fks[:, 0:8], in_=fk[:, 0:8],
                        func=mybir.ActivationFunctionType.Sqrt,
                        bias=eps_t, scale=1.0 / H)
                    nc.vector.reciprocal(out=fk[:, 0:8], in_=fks[:, 0:8])
                    for tt in range(8):
                        k_raw = pa[tt][:, 0:H]
                        nc.vector.scalar_tensor_tensor(
                            out=kn_pre[:, tt, :], in0=k_raw,
                            scalar=fk[:, tt:tt + 1], in1=kscale_sb,
                            op0=mybir.AluOpType.mult, op1=mybir.AluOpType.mult)

                # phase 1b: noise rows (t 1024:2048) -> q|k|v
                with tc.tile_pool(name="xh2", bufs=6) as xh2, \
                     tc.tile_pool(name="ppb", bufs=8, space="PSUM") as ppb, \
                     tc.tile_pool(name="sq2", bufs=2) as sqp2:
                    pb = [ppb.tile([P, 512], F32, tag="pb", name="pb") for _ in range(8)]
                    for dd in range(16):
                        xb = xh2.tile([P, T_NOISE], F32R, tag="x")
                        nc.sync.dma_start(
                            out=xb, in_=xT_d[dd * P:(dd + 1) * P, T_PAD:T_X])
                        wb = wp.tile([P, 512], F32R, tag="wall", name="wb")
                        nc.sync.dma_start(
                            out=wb, in_=w_all_d[dd * P:(dd + 1) * P, :])
                        for i in range(8):
                            nc.tensor.matmul(
                                pb[i],
                                lhsT=xb[:, i * P:(i + 1) * P],
                                rhs=wb,
                                start=(dd == 0), stop=(dd == 15))
                    for i in range(8):
                        tt = 8 + i
                        k_raw = pb[i][:, 256:384]
                        v_raw = pb[i][:, 384:512]
                        sq = sqp2.tile([P, H], F32, tag="sq")
                        nc.scalar.activation(
                            out=sq, in_=k_raw,
                            func=mybir.ActivationFunctionType.Square,
                            accum_out=fk[:, tt:tt + 1])
                        nc.scalar.copy(out=v_sb[:, tt, :], in_=v_raw)
                        nc.vector.tensor_copy(v_r[:, tt, :], v_raw)
                        for j in range(2):
                            q_raw = pb[i][:, j * H:(j + 1) * H]
                            sq2 = sqp2.tile([P, H], F32, tag="sq")
                            nc.scalar.activation(
                                out=sq2, in_=q_raw,
                                func=mybir.ActivationFunctionType.Square,
                                accum_out=fq[:, 2 * i + j:2 * i + j + 1])
                    nc.scalar.activation(
                        out=fks[:, 8:16], in_=fk[:, 8:16],
                        func=mybir.ActivationFunctionType.Sqrt,
                        bias=eps_t, scale=1.0 / H)
                    nc.vector.reciprocal(out=fk[:, 8:16], in_=fks[:, 8:16])
                    nc.scalar.activation(
                        out=fqs, in_=fq,
                        func=mybir.ActivationFunctionType.Sqrt,
                        bias=eps_t, scale=1.0 / H)
                    nc.vector.reciprocal(out=fq, in_=fqs)
                    for i in range(8):
                        tt = 8 + i
                        nc.vector.scalar_tensor_tensor(
                            out=kn_pre[:, tt, :], in0=pb[i][:, 256:384],
                            scalar=fk[:, tt:tt + 1], in1=kscale_sb,
                            op0=mybir.AluOpType.mult, op1=mybir.AluOpType.mult)
                        for j in range(2):
                            nc.vector.scalar_tensor_tensor(
                                out=qn_pre[:, i, j * H:(j + 1) * H],
                                in0=pb[i][:, j * H:(j + 1) * H],
                                scalar=fq[:, 2 * i + j:2 * i + j + 1],
                                in1=qscale_sb,
                                op0=mybir.AluOpType.mult, op1=mybir.AluOpType.mult)

                # ---- RoPE (rotate-half) ----
                cosk_sb = ps.tile([P, NTT, H], F32)
                nc.sync.dma_start(
                    out=cosk_sb, in_=cosk_d[:, :].rearrange("(tt p) h -> p tt h", p=P))
                sink_sb = ps.tile([P, NTT, H], F32)
                nc.sync.dma_start(
                    out=sink_sb, in_=sink_d[:, :].rearrange("(tt p) h -> p tt h", p=P))

                m2k = scr.tile([P, NTT, H], F32)
                nc.vector.tensor_mul(kn, kn_pre, cosk_sb)
                nc.vector.tensor_mul(
                    m2k[:, :, 0:HALF], kn_pre[:, :, HALF:H], sink_sb[:, :, 0:HALF])
                nc.vector.tensor_mul(
                    m2k[:, :, HALF:H], kn_pre[:, :, 0:HALF], sink_sb[:, :, HALF:H])
                nc.vector.tensor_add(kn, kn, m2k)

                m2q = scr.tile([P, NQT, 256], F32)
                for j in range(2):
                    o = j * H
                    nc.vector.tensor_mul(
                        qn[:, :, o:o + H], qn_pre[:, :, o:o + H],
                        cosk_sb[:, 8:16, :])
                    nc.vector.tensor_mul(
                        m2q[:, :, o:o + HALF], qn_pre[:, :, o + HALF:o + H],
                        sink_sb[:, 8:16, 0:HALF])
                    nc.vector.tensor_mul(
                        m2q[:, :, o + HALF:o + H], qn_pre[:, :, o:o + HALF],
                        sink_sb[:, 8:16, HALF:H])
                nc.vector.tensor_add(qn, qn, m2q)

            # k/v new -> DRAM (for host-side cache assembly)
            nc.sync.dma_start(
                out=kout_d[:, :].rearrange("(tt p) h -> p tt h", p=P), in_=kn)
            nc.sync.dma_start(
                out=vout_d[:, :].rearrange("(tt p) h -> p tt h", p=P), in_=v_sb)

            # ---- transposes: (t, h) -> (h, t) for q and k ----
            with tc.tile_pool(name="tp", bufs=4, space="PSUM") as tpp:
                for tt in range(NTT):
                    t_ps = tpp.tile([P, P], F32, tag="t")
                    nc.tensor.transpose(t_ps, kn[:, tt, :], ident)
                    nc.vector.tensor_copy(kT[:, tt * P:(tt + 1) * P], t_ps)
                for i in range(NQT):
                    for j in range(2):
                        t_ps = tpp.tile([P, P], F32, tag="t")
                        nc.tensor.transpose(t_ps, qn[:, i, j * H:(j + 1) * H], ident)
                        nc.vector.tensor_copy(qT[j][:, i * P:(i + 1) * P], t_ps)

            # old-cache K (pre-transposed on host) and V
            ktold_sb = ps.tile([P, 2, n_old * P], F32R)
            nc.sync.dma_start(
                out=ktold_sb, in_=ktold_d[:, :, :].rearrange("j p k -> p j k"))
            vold_sb = ps.tile([P, 2, n_old, H], F32R)
            nc.sync.dma_start(
                out=vold_sb,
                in_=vold_d[:, :, :].rearrange("j (kc p) h -> p j kc h", p=P))

            # ---- attention: scoresT -> exp -> attnT accumulate + denom ----
            def kv_chunk(j, ck):
                """(kT_slice [128h, 128kv], V_slice [128kv, 128h]) for chunk ck."""
                if ck < n_old:
                    return (ktold_sb[:, j, ck * P:(ck + 1) * P],
                            vold_sb[:, j, ck, :])
                if ck < n_old + n_ctx:
                    i = ck - n_old
                    return (kT[:, i * P:(i + 1) * P], v_r[:, i, :])
                i = ck - n_old - n_ctx
                return (kT[:, T_PAD + i * P:T_PAD + (i + 1) * P],
                        v_r[:, T_PAD // P + i, :])

            with tc.tile_pool(name="sp", bufs=2, space="PSUM") as sp, \
                 tc.tile_pool(name="ap", bufs=1, space="PSUM") as apl, \
                 tc.tile_pool(name="dp", bufs=1, space="PSUM") as dpl, \
                 tc.tile_pool(name="ex", bufs=5) as exl, \
                 tc.tile_pool(name="sm", bufs=2) as sml:
                for j in range(2):
                    attn_ps = apl.tile([P, 2, 512], F32, tag="a")
                    den_ps = dpl.tile([1, 2, 512], F32, tag="d")
                    for ck in range(n_kv):
                        kt_ap, v_ap = kv_chunk(j, ck)
                        sc = sp.tile([P, 2, 512], F32, tag="sc")
                        for qb in range(2):
                            nc.tensor.matmul(
                                sc[:, qb, :], lhsT=kt_ap,
                                rhs=qT[j][:, qb * 512:(qb + 1) * 512],
                                start=True, stop=True)
                        ex = exl.tile([P, 2, 512], F32R, tag="ex")
                        nc.scalar.activation(
                            out=ex, in_=sc,
                            func=mybir.ActivationFunctionType.Exp, scale=SM_SCALE)
                        for qb in range(2):
                            nc.tensor.matmul(
                                attn_ps[:, qb, :], lhsT=v_ap,
                                rhs=ex[:, qb, :],
                                start=(ck == 0), stop=(ck == n_kv - 1))
                            nc.tensor.matmul(
                                den_ps[:, qb, :], lhsT=ones_col,
                                rhs=ex[:, qb, :],
                                start=(ck == 0), stop=(ck == n_kv - 1))
                    # softmax normalize: attnT = attn_ps * (1/denom) broadcast
                    recip = sml.tile([1, 2, 512], F32R, tag="r")
                    with nc.allow_low_precision(
                            reason="softmax denom reciprocal in f32r"):
                        nc.vector.reciprocal(out=recip, in_=den_ps)
                    denb = sp.tile([P, 2, 512], F32, tag="sc")
                    for qb in range(2):
                        nc.tensor.matmul(
                            denb[:, qb, :], lhsT=ones_row,
                            rhs=recip[:, qb, :],
                            start=True, stop=True)
                    denb_sb = sml.tile([P, 2, 512], F32, tag="db")
                    nc.scalar.copy(out=denb_sb, in_=denb)
                    nc.vector.tensor_mul(attnT[j], attn_ps, denb_sb)

            # ---- o_proj partial: out[t, d] = sum_j attnT_j[:, t].T @ wo_j ----
            wo_sb = ps.tile([P, 2, D], F32R)
            nc.sync.dma_start(
                out=wo_sb, in_=wo_d[:, :].rearrange("(j p) d -> p j d", p=P))
            with tc.tile_pool(name="op", bufs=6, space="PSUM") as opl, \
                 tc.tile_pool(name="os", bufs=6) as osl:
                for i in range(NQT):
                    qb, r = divmod(i * P, 512)
                    for dc in range(4):
                        o_ps = opl.tile([P, 512], F32, tag="o")
                        for j in range(2):
                            nc.tensor.matmul(
                                o_ps,
                                lhsT=attnT[j][:, qb, r:r + P],
                                rhs=wo_sb[:, j, dc * 512:(dc + 1) * 512],
                                start=(j == 0), stop=(j == 1))
                        o_sb = osl.tile([P, 512], F32, tag="os")
                        if (i * 4 + dc) % 2 == 0:
                            nc.scalar.copy(out=o_sb, in_=o_ps)
                        else:
                            nc.vector.tensor_copy(o_sb, o_ps)
                        nc.sync.dma_start(
                            out=out_d[i * P:(i + 1) * P, dc * 512:(dc + 1) * 512],
                            in_=o_sb)

    return nc


def _rope_tables(ctx_positions, noise_positions):
    pos = np.concatenate([ctx_positions, noise_positions]).astype(np.float32)
    inv_freq = (1.0 / (THETA ** (np.arange(HALF, dtype=np.float32) / HALF))
                ).astype(np.float32)
    ang = pos[:, None] * inv_freq[None, :]
    cos = np.cos(ang).astype(np.float32)
    sin = np.sin(ang).astype(np.float32)
    cosk = np.concatenate([cos, cos], axis=1)
    sink = np.concatenate([-sin, sin], axis=1)
    return np.ascontiguousarray(cosk), np.ascontiguousarray(sink)


def kernel(x_noise, target_hidden, noise_positions, ctx_positions,
           kv_cache_k, kv_cache_v, cache_len, actual_ctx_count,
           Wq, Wk, Wv, Wo, q_scale, k_scale):
    from concourse.bass_utils import run_bass_kernel_spmd

    x_noise = np.asarray(x_noise, dtype=np.float32)
    target_hidden = np.asarray(target_hidden, dtype=np.float32)
    noise_positions = np.asarray(noise_positions)
    ctx_positions = np.asarray(ctx_positions)
    kv_cache_k = np.asarray(kv_cache_k, dtype=np.float32)
    kv_cache_v = np.asarray(kv_cache_v, dtype=np.float32)
    Wq = np.asarray(Wq, dtype=np.float32)
    Wk = np.asarray(Wk, dtype=np.float32)
    Wv = np.asarray(Wv, dtype=np.float32)
    Wo = np.asarray(Wo, dtype=np.float32)
    q_scale = np.asarray(q_scale, dtype=np.float32)
    k_scale = np.asarray(k_scale, dtype=np.float32)
    cache_len = int(cache_len)
    actx = int(actual_ctx_count)

    assert x_noise.shape == (T_NOISE, D) and target_hidden.shape == (T_PAD, D)
    assert cache_len % P == 0 and actx % P == 0
    assert cache_len + actx + T_NOISE <= MAX_KV
    n_old, n_ctx, n_noise = cache_len // P, actx // P, T_NOISE // P

    x_new = np.concatenate([target_hidden, x_noise], axis=0)
    xT = np.ascontiguousarray(x_new.T)
    cosk, sink = _rope_tables(ctx_positions, noise_positions)
    kscale_b = np.ascontiguousarray(np.tile(k_scale[None, :], (P, 1)))
    qscale_b = np.ascontiguousarray(np.tile(q_scale[None, :], (P, 1)))

    nc = build_bass(n_old, n_ctx, n_noise)

    in_maps = []
    for c in range(N_CORES):
        hs = [2 * c, 2 * c + 1]
        w_all = np.concatenate([
            Wq[:, hs, :].reshape(D, 256), Wk[:, c, :], Wv[:, c, :]], axis=1)
        in_maps.append({
            "xT": xT,
            "w_all": np.ascontiguousarray(w_all),
            "wo": np.ascontiguousarray(Wo[hs].reshape(256, D)),
            "ktold": np.ascontiguousarray(
                np.transpose(kv_cache_k[0, hs, :cache_len, :], (0, 2, 1))),
            "vold": np.ascontiguousarray(kv_cache_v[0, hs, :cache_len, :]),
            "cosk": cosk,
            "sink": sink,
            "kscale_b": kscale_b,
            "qscale_b": qscale_b,
        })

    res = run_bass_kernel_spmd(nc, in_maps, core_ids=list(range(N_CORES)))
    outs = res.results

    output = outs[0]["out_part"].copy()
    for c in range(1, N_CORES):
        output += outs[c]["out_part"]

    k_cache = kv_cache_k.copy()
    v_cache = kv_cache_v.copy()
    noise_start = cache_len + actx
    for c in range(N_CORES):
        k_new = outs[c]["k_out"]
        v_new = outs[c]["v_out"]
        for h in (2 * c, 2 * c + 1):
            k_cache[0, h, cache_len:cache_len + actx] = k_new[:actx]
            k_cache[0, h, noise_start:noise_start + T_NOISE] = k_new[T_PAD:]
            v_cache[0, h, cache_len:cache_len + actx] = v_new[:actx]
            v_cache[0, h, noise_start:noise_start + T_NOISE] = v_new[T_PAD:]

    return output, k_cache, v_cache


# revision 16
# speedup vs baseline: 1.0101x; 1.0101x over previous
"""DFlashAttention kernel for 8 TRN2 NeuronCores (Bass/Tile).

Sharding: tensor-parallel over heads. Core c owns query heads {2c, 2c+1}
and kv head c (GQA group). Each core computes its heads' full attention
and an o_proj partial; the host sums the 8 partials (the "all-reduce")
and assembles the updated KV caches from the per-core k/v projections.

Device layout notes:
  - x_new is fed pre-transposed (d, t) so projection matmuls can contract
    over d on the partition axis without any on-chip transpose of x.
  - q/k are computed in (t, h) layout (RMSNorm reduces over h on the free
    axis), then PE-transposed to (h, t) for the score matmuls.
  - scores are computed as scoresT[kv, q] so that exp(scores) tiles can be
    used directly as the moving operand of the attention matmul
    (attnT[h, q] += V[kv, h].T @ expT[kv, q]) and the softmax denominator
    is a ones-weight matmul (denom[1, q] += ones[kv, 1].T @ expT[kv, q]).
  - attnT[h, q] is already the stationary-operand layout for o_proj.
"""

import numpy as np

import concourse.bass as bass
import concourse.mybir as mybir
import concourse.tile as tile
from concourse.masks import make_identity
from concourse.vector_clock import ScopedClock

F32 = mybir.dt.float32
F32R = mybir.dt.float32r


class SplitDrainTileContext(tile.TileContext):
    """TileContext that caps every instruction at one sem wait.

    The walrus build in this container rejects instructions with more than
    one sync-wait command (CoreV3GenImpl setupSyncWait: "Too many sync wait
    commands"). Tile routinely assigns several waits to one instruction.
    Semantics are preserved by moving all but the last wait onto same-engine
    NoOps inserted immediately before the instruction — each engine
    sequencer executes its queue in order, so waiting serially on N nops is
    equivalent to one instruction waiting on N semaphores.
    """

    _N_SPLIT_NOPS = 40

    def _add_instruction(self, inst):
        si = inst.sync_info
        if si is not None and si.on_wait is not None and len(si.on_wait) > 1:
            import bass_rust
            waits = list(si.on_wait)
            for w in waits[:-1]:
                nop = bass_rust.InstNoOp(
                    name=self.nc.get_next_instruction_name(), ins=[], outs=[])
                nop.engine = inst.engine
                nop.sync_info = bass_rust.SyncInfo(on_wait=[w], on_update=[])
                super()._add_instruction(nop)
            inst.sync_info = bass_rust.SyncInfo(
                on_wait=[waits[-1]], on_update=list(si.on_update or []))
        super()._add_instruction(inst)

    def _drain_and_barrier(self, tick_clock, wait_clock):
        nops = [
            self.nc.sync.nop(nofuse=True, hint="tail_drain_wait")
            for _ in range(self._N_SPLIT_NOPS)
        ]
        drain_inst = self.nc.sync.drain()
        wait_clock.add_sem_waits(
            drain_inst.ins, ScopedClock({None: tick_clock.global_clock})
        )
        si = drain_inst.ins.sync_info
        waits = list(si.on_wait or [])
        if len(waits) > 1:
            assert len(waits) <= self._N_SPLIT_NOPS + 1, len(waits)
            import bass_rust
            for nop, w in zip(nops, waits[:-1]):
                nop.ins.sync_info = bass_rust.SyncInfo(on_wait=[w], on_update=[])
            drain_inst.ins.sync_info = bass_rust.SyncInfo(
                on_wait=[waits[-1]], on_update=list(si.on_update or []))

        self.nc.all_engine_barrier()
        assert self.sems is not None
        popped = self.nc._tile_sem_poison_stack.pop()
        assert popped is self._sem_poison
        self.nc.clear_and_free_semaphores(list(self.sems.allocated().values()))
        self.nc.all_engine_barrier()

# Problem constants (hardcoded; see module docstring).
D = 2048          # hidden size
N_HEADS = 16      # query heads
K_HEADS = 8       # kv heads
H = 128           # head dim
HALF = H // 2
T_NOISE = 1024
T_PAD = 1024
T_X = T_PAD + T_NOISE          # rows of x_new
MAX_KV = 4096
THETA = 1000000.0
EPS = 1e-6
SM_SCALE = H ** -0.5
N_CORES = 8
P = 128           # partitions

NTT = T_X // P                 # 16 t-tiles over x_new
NQT = T_NOISE // P             # 8 t-tiles over x_noise


def build_bass(n_old: int, n_ctx: int, n_noise: int) -> bass.Bass:
    """Build the per-core Bass module (SPMD: same IR on all 8 cores).

    n_old   = cache_len // 128        (old-cache kv chunks)
    n_ctx   = actual_ctx_count // 128 (valid ctx kv chunks)
    n_noise = T_NOISE // 128          (noise kv chunks)
    """
    nc = bass.Bass()

    # ---- I/O ----
    xT_d = nc.declare_dram_parameter("xT", (D, T_X), F32R, isOutput=False)
    w_all_d = nc.declare_dram_parameter("w_all", (D, 512), F32R, isOutput=False)
    wo_d = nc.declare_dram_parameter("wo", (256, D), F32R, isOutput=False)
    ktold_d = nc.declare_dram_parameter(
        "ktold", (2, H, n_old * P), F32R, isOutput=False)
    vold_d = nc.declare_dram_parameter(
        "vold", (2, n_old * P, H), F32R, isOutput=False)
    cosk_d = nc.declare_dram_parameter("cosk", (T_X, H), F32, isOutput=False)
    sink_d = nc.declare_dram_parameter("sink", (T_X, H), F32, isOutput=False)
    kscale_d = nc.declare_dram_parameter("kscale_b", (P, H), F32, isOutput=False)
    qscale_d = nc.declare_dram_parameter("qscale_b", (P, H), F32, isOutput=False)

    out_d = nc.declare_dram_parameter("out_part", (T_NOISE, D), F32, isOutput=True)
    kout_d = nc.declare_dram_parameter("k_out", (T_X, H), F32, isOutput=True)
    vout_d = nc.declare_dram_parameter("v_out", (T_X, H), F32, isOutput=True)

    n_kv = n_old + n_ctx + n_noise   # total kv chunks (22)

    with SplitDrainTileContext(nc) as tc:
        with tc.tile_pool(name="persist", bufs=1) as ps:
            # ---- persistent SBUF tiles ----
            ident = ps.tile([P, P], F32)
            make_identity(nc, ident)
            ones_f32 = ps.tile([P, P], F32)
            nc.vector.memset(ones_f32, 1.0)
            ones_col = ps.tile([P, 1], F32R)
            nc.vector.tensor_copy(ones_col, ones_f32[:, 0:1])
            ones_row = ps.tile([1, P], F32R)
            nc.vector.tensor_copy(ones_row, ones_f32[0:1, :])
            eps_t = ps.tile([P, 1], F32)
            nc.vector.memset(eps_t, EPS)

            kscale_sb = ps.tile([P, H], F32)
            nc.sync.dma_start(out=kscale_sb, in_=kscale_d[:, :])
            qscale_sb = ps.tile([P, H], F32)
            nc.sync.dma_start(out=qscale_sb, in_=qscale_d[:, :])

            kn = ps.tile([P, NTT, H], F32)       # roped+normed k (t, h)
            v_sb = ps.tile([P, NTT, H], F32)     # raw v (t, h), exact for v_out
            v_r = ps.tile([P, NTT, H], F32R)    # rounded v for matmul
            qn = ps.tile([P, NQT, 256], F32)     # roped+normed q (t, 2 heads)
            kT = ps.tile([P, T_X], F32R)          # k transposed (h, t)
            qT = [ps.tile([P, T_NOISE], F32R, tag=f"qT{j}", name=f"qT{j}") for j in range(2)]
            attnT = [ps.tile([P, 2, 512], F32R, tag=f"attnT{j}", name=f"attnT{j}") for j in range(2)]

            # ---- phase 1: projections q|k|v = x_new @ W ----
            with tc.tile_pool(name="wp", bufs=4) as wp, \
                 tc.tile_pool(name="scr", bufs=1) as scr:
                kn_pre = scr.tile([P, NTT, H], F32)
                qn_pre = scr.tile([P, NQT, 256], F32)
                f_all = scr.tile([P, 64], F32)
                fk = f_all[:, 0:16]
                fq = f_all[:, 16:32]
                fks = f_all[:, 32:48]
                fqs = f_all[:, 48:64]

                # phase 1a: ctx rows (t 0:1024) -> k|v only
                with tc.tile_pool(name="xh", bufs=6) as xh, \
                     tc.tile_pool(name="ppa", bufs=8, space="PSUM") as ppa, \
                     tc.tile_pool(name="sq", bufs=2) as sqp:
                    pa = [ppa.tile([P, 256], F32, tag="pa", name="pa") for _ in range(8)]
                    for dd in range(16):
                        xa = xh.tile([P, T_PAD], F32R, tag="x")
                        nc.sync.dma_start(
                            out=xa, in_=xT_d[dd * P:(dd + 1) * P, 0:T_PAD])
                        wa = wp.tile([P, 256], F32R, tag="wkv", name="wa")
                        nc.sync.dma_start(
                            out=wa, in_=w_all_d[dd * P:(dd + 1) * P, 256:512])
                        for tt in range(8):
                            nc.tensor.matmul(
                                pa[tt],
                                lhsT=xa[:, tt * P:(tt + 1) * P],
                                rhs=wa,
                                start=(dd == 0), stop=(dd == 15))
                    # drain: rmsnorm stats + v copies for ctx tiles
                    for tt in range(8):
                        k_raw = pa[tt][:, 0:H]
                        v_raw = pa[tt][:, H:256]
                        sq = sqp.tile([P, H], F32, tag="sq")
                        nc.scalar.activation(
                            out=sq, in_=k_raw,
                            func=mybir.ActivationFunctionType.Square,
                            accum_out=fk[:, tt:tt + 1])
                        nc.scalar.copy(out=v_sb[:, tt, :], in_=v_raw)
                        nc.vector.tensor_copy(v_r[:, tt, :], v_raw)
                    nc.scalar.activation(
                        out=fks[:, 0:8], in_=fk[:, 0:8],
                        func=mybir.ActivationFunctionType.Sqrt,
                        bias=eps_t, scale=1.0 / H)
                    nc.vector.reciprocal(out=fk[:, 0:8], in_=fks[:, 0:8])
                    for tt in range(8):
                        k_raw = pa[tt][:, 0:H]
                        nc.vector.scalar_tensor_tensor(
                            out=kn_pre[:, tt, :], in0=k_raw,
                            scalar=fk[:, tt:tt + 1], in1=kscale_sb,
                            op0=mybir.AluOpType.mult, op1=mybir.AluOpType.mult)

                cosk_sb = ps.tile([P, NTT, H], F32)
                nc.sync.dma_start(
                    out=cosk_sb, in_=cosk_d[:, :].rearrange("(tt p) h -> p tt h", p=P))
                sink_sb = ps.tile([P, NTT, H], F32)
                nc.sync.dma_start(
                    out=sink_sb, in_=sink_d[:, :].rearrange("(tt p) h -> p tt h", p=P))

                # phase 1b: noise rows (t 1024:2048) -> q|k|v
                with tc.tile_pool(name="xh2", bufs=6) as xh2, \
                     tc.tile_pool(name="ppb", bufs=8, space="PSUM") as ppb, \
                     tc.tile_pool(name="sq2", bufs=2) as sqp2:
                    pb = [ppb.tile([P, 512], F32, tag="pb", name="pb") for _ in range(8)]
                    for dd in range(16):
                        xb = xh2.tile([P, T_NOISE], F32R, tag="x")
                        nc.sync.dma_start(
                            out=xb, in_=xT_d[dd * P:(dd + 1) * P, T_PAD:T_X])
                        wb = wp.tile([P, 512], F32R, tag="wall", name="wb")
                        nc.sync.dma_start(
                            out=wb, in_=w_all_d[dd * P:(dd + 1) * P, :])
                        for i in range(8):
                            nc.tensor.matmul(
                                pb[i],
                                lhsT=xb[:, i * P:(i + 1) * P],
                                rhs=wb,
                                start=(dd == 0), stop=(dd == 15))
                    for i in range(8):
                        tt = 8 + i
                        k_raw = pb[i][:, 256:384]
                        v_raw = pb[i][:, 384:512]
                        sq = sqp2.tile([P, H], F32, tag="sq")
                        nc.scalar.activation(
                            out=sq, in_=k_raw,
                            func=mybir.ActivationFunctionType.Square,
                            accum_out=fk[:, tt:tt + 1])
                        nc.scalar.copy(out=v_sb[:, tt, :], in_=v_raw)
                        nc.vector.tensor_copy(v_r[:, tt, :], v_raw)
                        for j in range(2):
                            q_raw = pb[i][:, j * H:(j + 1) * H]
                            sq2 = sqp2.tile([P, H], F32, tag="sq")
                            nc.scalar.activation(
                                out=sq2, in_=q_raw,
                                func=mybir.ActivationFunctionType.Square,
                                accum_out=fq[:, 2 * i + j:2 * i + j + 1])
                    nc.scalar.activation(
                        out=fks[:, 8:16], in_=fk[:, 8:16],
                        func=mybir.ActivationFunctionType.Sqrt,
                        bias=eps_t, scale=1.0 / H)
                    nc.vector.reciprocal(out=fk[:, 8:16], in_=fks[:, 8:16])
                    nc.scalar.activation(
                        out=fqs, in_=fq,
                        func=mybir.ActivationFunctionType.Sqrt,
                        bias=eps_t, scale=1.0 / H)
                    nc.vector.reciprocal(out=fq, in_=fqs)
                    for i in range(8):
                        for j in range(2):
                            nc.vector.scalar_tensor_tensor(
                                out=qn_pre[:, i, j * H:(j + 1) * H],
                                in0=pb[i][:, j * H:(j + 1) * H],
                                scalar=fq[:, 2 * i + j:2 * i + j + 1],
                                in1=qscale_sb,
                                op0=mybir.AluOpType.mult, op1=mybir.AluOpType.mult)
                    for i in range(8):
                        tt = 8 + i
                        nc.vector.scalar_tensor_tensor(
                            out=kn_pre[:, tt, :], in0=pb[i][:, 256:384],
                            scalar=fk[:, tt:tt + 1], in1=kscale_sb,
                            op0=mybir.AluOpType.mult, op1=mybir.AluOpType.mult)

                # ---- RoPE (rotate-half), q first so attention can start ----
                m2q = scr.tile([P, NQT, 256], F32)
                for j in range(2):
                    o = j * H
                    nc.vector.tensor_mul(
                        qn[:, :, o:o + H], qn_pre[:, :, o:o + H],
                        cosk_sb[:, 8:16, :])
                    nc.vector.tensor_mul(
                        m2q[:, :, o:o + HALF], qn_pre[:, :, o + HALF:o + H],
                        sink_sb[:, 8:16, 0:HALF])
                    nc.vector.tensor_mul(
                        m2q[:, :, o + HALF:o + H], qn_pre[:, :, o:o + HALF],
                        sink_sb[:, 8:16, HALF:H])
                nc.vector.tensor_add(qn, qn, m2q)

                # q transposes on PE (psum scratch), cast copies split ACT/DVE
                with tc.tile_pool(name="tpq", bufs=4, space="PSUM") as tpq:
                    for i in range(NQT):
                        for j in range(2):
                            t_ps = tpq.tile([P, P], F32, tag="t")
                            nc.tensor.transpose(
                                t_ps, qn[:, i, j * H:(j + 1) * H], ident)
                            if (i + j) % 2 == 0:
                                nc.vector.tensor_copy(
                                    qT[j][:, i * P:(i + 1) * P], t_ps)
                            else:
                                nc.scalar.copy(
                                    out=qT[j][:, i * P:(i + 1) * P], in_=t_ps)

                # k rope: ctx half on DVE, noise half on GpSimd (parallel)
                m2k = scr.tile([P, NTT, H], F32)
                nc.vector.tensor_mul(
                    kn[:, 0:8, :], kn_pre[:, 0:8, :], cosk_sb[:, 0:8, :])
                nc.vector.tensor_mul(
                    m2k[:, 0:8, 0:HALF], kn_pre[:, 0:8, HALF:H],
                    sink_sb[:, 0:8, 0:HALF])
                nc.vector.tensor_mul(
                    m2k[:, 0:8, HALF:H], kn_pre[:, 0:8, 0:HALF],
                    sink_sb[:, 0:8, HALF:H])
                nc.vector.tensor_add(kn[:, 0:8, :], kn[:, 0:8, :], m2k[:, 0:8, :])
                nc.gpsimd.tensor_mul(
                    kn[:, 8:16, :], kn_pre[:, 8:16, :], cosk_sb[:, 8:16, :])
                nc.gpsimd.tensor_mul(
                    m2k[:, 8:16, 0:HALF], kn_pre[:, 8:16, HALF:H],
                    sink_sb[:, 8:16, 0:HALF])
                nc.gpsimd.tensor_mul(
                    m2k[:, 8:16, HALF:H], kn_pre[:, 8:16, 0:HALF],
                    sink_sb[:, 8:16, HALF:H])
                nc.gpsimd.tensor_add(
                    kn[:, 8:16, :], kn[:, 8:16, :], m2k[:, 8:16, :])

                # k transposes on PE
                with tc.tile_pool(name="tpk", bufs=4, space="PSUM") as tpk:
                    for tt in range(NTT):
                        t_ps = tpk.tile([P, P], F32, tag="t")
                        nc.tensor.transpose(t_ps, kn[:, tt, :], ident)
                        if tt % 2 == 0:
                            nc.vector.tensor_copy(
                                kT[:, tt * P:(tt + 1) * P], t_ps)
                        else:
                            nc.scalar.copy(
                                out=kT[:, tt * P:(tt + 1) * P], in_=t_ps)

            # k/v new -> DRAM (for host-side cache assembly)
            nc.sync.dma_start(
                out=kout_d[:, :].rearrange("(tt p) h -> p tt h", p=P), in_=kn)
            nc.sync.dma_start(
                out=vout_d[:, :].rearrange("(tt p) h -> p tt h", p=P), in_=v_sb)

            # old-cache K (pre-transposed on host) and V
            ktold_sb = ps.tile([P, 2, n_old * P], F32R)
            nc.sync.dma_start(
                out=ktold_sb, in_=ktold_d[:, :, :].rearrange("j p k -> p j k"))
            vold_sb = ps.tile([P, 2, n_old, H], F32R)
            nc.sync.dma_start(
                out=vold_sb,
                in_=vold_d[:, :, :].rearrange("j (kc p) h -> p j kc h", p=P))

            # ---- attention: scoresT -> exp -> attnT accumulate + denom ----
            def kv_chunk(j, ck):
                """(kT_slice [128h, 128kv], V_slice [128kv, 128h]) for chunk ck."""
                if ck < n_old:
                    return (ktold_sb[:, j, ck * P:(ck + 1) * P],
                            vold_sb[:, j, ck, :])
                if ck < n_old + n_ctx:
                    i = ck - n_old
                    return (kT[:, i * P:(i + 1) * P], v_r[:, i, :])
                i = ck - n_old - n_ctx
                return (kT[:, T_PAD + i * P:T_PAD + (i + 1) * P],
                        v_r[:, T_PAD // P + i, :])

            with tc.tile_pool(name="sp", bufs=2, space="PSUM") as sp, \
                 tc.tile_pool(name="ap", bufs=1, space="PSUM") as apl, \
                 tc.tile_pool(name="dp", bufs=1, space="PSUM") as dpl, \
                 tc.tile_pool(name="ex", bufs=5) as exl, \
                 tc.tile_pool(name="sm", bufs=2) as sml:
                for j in range(2):
                    attn_ps = apl.tile([P, 2, 512], F32, tag="a")
                    den_ps = dpl.tile([1, 2, 512], F32, tag="d")
                    for ck in range(n_kv):
                        kt_ap, v_ap = kv_chunk(j, ck)
                        sc = sp.tile([P, 2, 512], F32, tag="sc")
                        for qb in range(2):
                            nc.tensor.matmul(
                                sc[:, qb, :], lhsT=kt_ap,
                                rhs=qT[j][:, qb * 512:(qb + 1) * 512],
                                start=True, stop=True)
                        ex = exl.tile([P, 2, 512], F32R, tag="ex")
                        nc.scalar.activation(
                            out=ex, in_=sc,
                            func=mybir.ActivationFunctionType.Exp, scale=SM_SCALE)
                        for qb in range(2):
                            nc.tensor.matmul(
                                attn_ps[:, qb, :], lhsT=v_ap,
                                rhs=ex[:, qb, :],
                                start=(ck == 0), stop=(ck == n_kv - 1))
                            nc.tensor.matmul(
                                den_ps[:, qb, :], lhsT=ones_col,
                                rhs=ex[:, qb, :],
                                start=(ck == 0), stop=(ck == n_kv - 1))
                    # softmax normalize: broadcast denom via rank-1 matmul,
                    # then a full-width reciprocal (128 lanes, not 1)
                    den_sb = sml.tile([1, 2, 512], F32R, tag="r")
                    nc.scalar.copy(out=den_sb, in_=den_ps)
                    denb = sp.tile([P, 2, 512], F32, tag="sc")
                    for qb in range(2):
                        nc.tensor.matmul(
                            denb[:, qb, :], lhsT=ones_row,
                            rhs=den_sb[:, qb, :],
                            start=True, stop=True)
                    denb_sb = sml.tile([P, 2, 512], F32, tag="db")
                    nc.vector.reciprocal(out=denb_sb, in_=denb)
                    nc.vector.tensor_mul(attnT[j], attn_ps, denb_sb)

            # ---- o_proj partial: out[t, d] = sum_j attnT_j[:, t].T @ wo_j ----
            wo_sb = ps.tile([P, 2, D], F32R)
            nc.sync.dma_start(
                out=wo_sb, in_=wo_d[:, :].rearrange("(j p) d -> p j d", p=P))
            with tc.tile_pool(name="op", bufs=6, space="PSUM") as opl, \
                 tc.tile_pool(name="os", bufs=6) as osl:
                for i in range(NQT):
                    qb, r = divmod(i * P, 512)
                    for dc in range(4):
                        o_ps = opl.tile([P, 512], F32, tag="o")
                        for j in range(2):
                            nc.tensor.matmul(
                                o_ps,
                                lhsT=attnT[j][:, qb, r:r + P],
                                rhs=wo_sb[:, j, dc * 512:(dc + 1) * 512],
                                start=(j == 0), stop=(j == 1))
                        o_sb = osl.tile([P, 512], F32, tag="os")
                        if (i * 4 + dc) % 2 == 0:
                            nc.scalar.copy(out=o_sb, in_=o_ps)
                        else:
                            nc.vector.tensor_copy(o_sb, o_ps)
                        nc.sync.dma_start(
                            out=out_d[i * P:(i + 1) * P, dc * 512:(dc + 1) * 512],
                            in_=o_sb)

    return nc


def _rope_tables(ctx_positions, noise_positions):
    pos = np.concatenate([ctx_positions, noise_positions]).astype(np.float32)
    inv_freq = (1.0 / (THETA ** (np.arange(HALF, dtype=np.float32) / HALF))
                ).astype(np.float32)
    ang = pos[:, None] * inv_freq[None, :]
    cos = np.cos(ang).astype(np.float32)
    sin = np.sin(ang).astype(np.float32)
    cosk = np.concatenate([cos, cos], axis=1)
    sink = np.concatenate([-sin, sin], axis=1)
    return np.ascontiguousarray(cosk), np.ascontiguousarray(sink)


def kernel(x_noise, target_hidden, noise_positions, ctx_positions,
           kv_cache_k, kv_cache_v, cache_len, actual_ctx_count,
           Wq, Wk, Wv, Wo, q_scale, k_scale):
    from concourse.bass_utils import run_bass_kernel_spmd

    x_noise = np.asarray(x_noise, dtype=np.float32)
    target_hidden = np.asarray(target_hidden, dtype=np.float32)
    noise_positions = np.asarray(noise_positions)
    ctx_positions = np.asarray(ctx_positions)
    kv_cache_k = np.asarray(kv_cache_k, dtype=np.float32)
    kv_cache_v = np.asarray(kv_cache_v, dtype=np.float32)
    Wq = np.asarray(Wq, dtype=np.float32)
    Wk = np.asarray(Wk, dtype=np.float32)
    Wv = np.asarray(Wv, dtype=np.float32)
    Wo = np.asarray(Wo, dtype=np.float32)
    q_scale = np.asarray(q_scale, dtype=np.float32)
    k_scale = np.asarray(k_scale, dtype=np.float32)
    cache_len = int(cache_len)
    actx = int(actual_ctx_count)

    assert x_noise.shape == (T_NOISE, D) and target_hidden.shape == (T_PAD, D)
    assert cache_len % P == 0 and actx % P == 0
    assert cache_len + actx + T_NOISE <= MAX_KV
    n_old, n_ctx, n_noise = cache_len // P, actx // P, T_NOISE // P

    x_new = np.concatenate([target_hidden, x_noise], axis=0)
    xT = np.ascontiguousarray(x_new.T)
    cosk, sink = _rope_tables(ctx_positions, noise_positions)
    kscale_b = np.ascontiguousarray(np.tile(k_scale[None, :], (P, 1)))
    qscale_b = np.ascontiguousarray(np.tile(q_scale[None, :], (P, 1)))

    nc = build_bass(n_old, n_ctx, n_noise)

    in_maps = []
    for c in range(N_CORES):
        hs = [2 * c, 2 * c + 1]
        w_all = np.concatenate([
            Wq[:, hs, :].reshape(D, 256), Wk[:, c, :], Wv[:, c, :]], axis=1)
        in_maps.append({
            "xT": xT,
            "w_all": np.ascontiguousarray(w_all),
            "wo": np.ascontiguousarray(Wo[hs].reshape(256, D)),
            "ktold": np.ascontiguousarray(
                np.transpose(kv_cache_k[0, hs, :cache_len, :], (0, 2, 1))),
            "vold": np.ascontiguousarray(kv_cache_v[0, hs, :cache_len, :]),
            "cosk": cosk,
            "sink": sink,
            "kscale_b": kscale_b,
            "qscale_b": qscale_b,
        })

    res = run_bass_kernel_spmd(nc, in_maps, core_ids=list(range(N_CORES)))
    outs = res.results

    output = outs[0]["out_part"].copy()
    for c in range(1, N_CORES):
        output += outs[c]["out_part"]

    k_cache = kv_cache_k.copy()
    v_cache = kv_cache_v.copy()
    noise_start = cache_len + actx
    for c in range(N_CORES):
        k_new = outs[c]["k_out"]
        v_new = outs[c]["v_out"]
        for h in (2 * c, 2 * c + 1):
            k_cache[0, h, cache_len:cache_len + actx] = k_new[:actx]
            k_cache[0, h, noise_start:noise_start + T_NOISE] = k_new[T_PAD:]
            v_cache[0, h, cache_len:cache_len + actx] = v_new[:actx]
            v_cache[0, h, noise_start:noise_start + T_NOISE] = v_new[T_PAD:]

    return output, k_cache, v_cache
